# revision 4
# baseline (speedup 1.0000x reference)
"""Trainium2 Bass kernel for masked attention scoring (sparse_attention).

Computes, per batch b:
    proj = y @ M^T                      # [B, D]
    eij  = tanh(einsum('bsd,bd->bs', x, proj))
    a    = exp(eij) * mask
    a    = a / (sum_s a + EPS)

Sharding: data-parallel over batch B=32 across 8 NeuronCores (4 batches
per core). M is replicated; all reductions stay local per shard.

v2 design (memory-bound; x stream is the roofline at ~430-490 GB/s):
  - x ships f16, split per batch into a host-TRANSPOSED part xt (j-chunks
    0..NT-1, laid out [e, s] so TensorE can ingest it as weights) and a
    natural part xn (chunks NT..15) for the DVE/ACT multiply+reduce path.
    All engines stay well under the DMA stream time so the 16 DMA queues
    run at their free rate instead of being consumer-throttled (the
    baseline's failure mode: 225 GB/s effective vs 490 free-run).
  - PE path: eij[s,.] columns accumulate in PSUM via 128x128 f16
    LDWEIGHTS tiles (lhsT=xT chunk) x projT column (rhs), 8 e-chunks per
    column. eij lands on 128 partitions, so the epilogue stays fast.
  - DVE/ACT path: per chunk either a fused scalar_tensor_tensor
    (mult,mult,accum) on DVE, or a 2x-mode paired tensor_mul + ACT
    Copy-accum, balancing DVE vs ACT busy time.
  - mask ships host-pre-transposed [P, BL, J] f32 (no device transposes),
    proj GEMM + projT transposes + partition-broadcast all on PE early,
    PE clock pre-warmed during the M DMA.
  - per-batch epilogue (tanh/exp/mask-mul/reduce) pipelines under the
    stream; only the final normalize + PE transpose + out DMA trail.
"""

import os
import sys

import numpy as np

for _p in ("/opt/trn_rl_repo",):
    if os.path.isdir(_p) and _p not in sys.path:
        sys.path.insert(0, _p)

B, S, D = 32, 2048, 1024
NCORES = 8
BL = B // NCORES        # batches per core
P = 128                 # SBUF partitions
J = S // P              # 16 s-chunks of 128 per batch
DC = D // P             # 8 e-chunks of 128
NT = 9                  # PE-path s-chunks per batch (j = 0..NT-1)
NN = J - NT             # natural-path s-chunks (j = NT..J-1)
N_STT = 3               # of NN: chunks done as fused STT on DVE
ST = NT * P
SN = NN * P
EPS = 1e-7

_CACHE = {}


def _build():
    import concourse.bacc as bacc
    import concourse.bass as bass_mod
    import concourse.tile as tile
    from concourse import mybir
    from concourse.masks import make_identity

    f32 = mybir.dt.float32
    f16 = mybir.dt.float16

    nc = bacc.Bacc("TRN2", target_bir_lowering=False, debug=False,
                   num_devices=NCORES)

    xt_ext = nc.dram_tensor("xt16", [BL, D, ST], f16, kind="ExternalInput").ap()
    xn_ext = nc.dram_tensor("xn16", [BL, SN, D], f16, kind="ExternalInput").ap()
    y_ext = nc.dram_tensor("yT16", [D, BL], f16, kind="ExternalInput").ap()
    m_ext = nc.dram_tensor("MT16", [D, D], f16, kind="ExternalInput").ap()
    mk_ext = nc.dram_tensor("maskT", [P, BL, J], f32, kind="ExternalInput").ap()
    out_ext = nc.dram_tensor("out", [BL, S], f32, kind="ExternalOutput").ap()

    with tile.TileContext(nc) as tc:
        with (
            tc.tile_pool(name="consts", bufs=1) as consts,
            tc.tile_pool(name="psum_proj", bufs=1, space="PSUM") as psum_proj,
            tc.tile_pool(name="psum_eij", bufs=1, space="PSUM") as psum_eij,
            tc.tile_pool(name="psum_pb", bufs=2, space="PSUM") as psum_pb,
            tc.tile_pool(name="psum_small", bufs=1, space="PSUM") as psum_small,
            tc.tile_pool(name="scr", bufs=6) as scr_pool,
        ):
            identity16 = consts.tile([P, P], f16)
            make_identity(nc, identity16)
            identity32 = consts.tile([P, P], f32)
            make_identity(nc, identity32)
            ones_col = consts.tile([P, 1], f32)
            nc.vector.memset(ones_col, 1.0)
            ones_row = consts.tile([1, P], f32)
            nc.vector.memset(ones_row, 1.0)
            eps_t = consts.tile([1, 1], f32)
            nc.vector.memset(eps_t, EPS)

            # ---- M^T ships pre-transposed f16 (contiguous 2KB rows) ----
            mtsb = consts.tile([P, DC, D], f16)
            m_src = m_ext.rearrange("(dc p) e -> p dc e", p=P)
            nc.sync.dma_start(out=mtsb[:, 0:DC // 2, :],
                              in_=m_src[:, 0:DC // 2, :])
            nc.sync.dma_start(out=mtsb[:, DC // 2:, :],
                              in_=m_src[:, DC // 2:, :])

            # warm the PE clock (1.2 -> 2.4 GHz needs ~3us sustained)
            warm_ps = psum_small.tile([P, P], f16, tag="small")
            for _ in range(12):
                nc.tensor.transpose(warm_ps, identity16, identity16)

            # ---- small inputs ----
            yT = consts.tile([P, DC, BL], f16)
            nc.sync.dma_start(
                out=yT, in_=y_ext.rearrange("(dc p) b -> p dc b", p=P))
            mask_all = consts.tile([P, BL, J], f32)
            nc.sync.dma_start(out=mask_all, in_=mk_ext)

            # ---- x: issue every DMA up front; whole shard is SBUF-resident
            xt_tiles = []
            xn_tiles = []
            for b in range(BL):
                xt_tiles.append(consts.tile([P, DC, ST], f16, name=f"xt{b}"))
                xn_tiles.append(consts.tile([P, NN, D], f16, name=f"xn{b}"))
            for b in range(BL):
                nc.sync.dma_start(
                    out=xt_tiles[b],
                    in_=xt_ext[b].rearrange("(ec p) s -> p ec s", p=P))
                xn_src = xn_ext[b].rearrange("(i p) e -> p i e", p=P)
                nc.sync.dma_start(out=xn_tiles[b][:, 0:N_STT, :],
                                  in_=xn_src[:, 0:N_STT, :])
                nc.sync.dma_start(out=xn_tiles[b][:, N_STT:, :],
                                  in_=xn_src[:, N_STT:, :])

            # ---- proj[b, e] = sum_d y[b, d] * M[e, d]  (PSUM f32) ----
            proj_ps = psum_proj.tile([BL, D], f32)
            for dc in range(DC):
                for eh in range(2):
                    nc.tensor.matmul(
                        proj_ps[:, eh * 512:(eh + 1) * 512],
                        lhsT=yT[:, dc, :],
                        rhs=mtsb[:, dc, eh * 512:(eh + 1) * 512],
                        start=(dc == 0),
                        stop=(dc == DC - 1),
                    )
            proj_sb = consts.tile([BL, D], f16)
            nc.scalar.copy(proj_sb[:, 0:512], proj_ps[:, 0:512])
            nc.scalar.copy(proj_sb[:, 512:], proj_ps[:, 512:])

            # ---- projT[p, ec, b] = proj[b, ec*128+p] via PE transposes ----
            projT = consts.tile([P, DC, BL], f16)
            for ec in range(DC):
                tp_ps = psum_small.tile([P, BL], f16, tag="ptr", bufs=2)
                nc.tensor.transpose(
                    tp_ps, proj_sb[:, ec * P:(ec + 1) * P],
                    identity16[:BL, :BL])
                nc.scalar.copy(projT[:, ec, :], tp_ps)

            # ---- broadcast proj rows across partitions (DVE path) ----
            projbc = []
            for b in range(BL):
                sel = consts.tile([BL, P], f16, name=f"sel{b}")
                nc.gpsimd.memset(sel, 0.0)
                nc.gpsimd.affine_select(
                    out=sel, in_=sel,
                    compare_op=mybir.AluOpType.not_equal,
                    fill=1.0, base=-b,
                    pattern=[[0, P]], channel_multiplier=1)
                pb = consts.tile([P, D], f16, name=f"projbc{b}")
                for eh in range(2):
                    pb_ps = psum_pb.tile([P, 512], f32, tag="pbps")
                    nc.tensor.matmul(
                        pb_ps,
                        lhsT=sel,
                        rhs=proj_sb[:, eh * 512:(eh + 1) * 512],
                        start=True, stop=True)
                    if eh == 0:
                        nc.vector.tensor_copy(pb[:, eh * 512:(eh + 1) * 512],
                                              pb_ps)
                    else:
                        nc.scalar.copy(pb[:, eh * 512:(eh + 1) * 512], pb_ps)
                projbc.append(pb)

            # ---- main pass ----
            eij_ps = psum_eij.tile([P, BL, NT], f32)
            eij_n = consts.tile([P, BL, NN], f32)
            th = consts.tile([P, BL, J], f32)
            ex = consts.tile([P, BL, J], f32)
            au = consts.tile([P, BL, J], f32)
            cs = consts.tile([P, BL], f32)

            for b in range(BL):
                xt = xt_tiles[b]
                xn = xn_tiles[b]
                # PE path: chunks j = 0..NT-1
                for j in range(NT):
                    for ec in range(DC):
                        nc.tensor.matmul(
                            eij_ps[:, b, j:j + 1],
                            lhsT=xt[:, ec, j * P:(j + 1) * P],
                            rhs=projT[:, ec, b:b + 1],
                            start=(ec == 0),
                            stop=(ec == DC - 1),
                        )
                # DVE fused STT path: chunks i = 0..N_STT-1
                for i in range(N_STT):
                    scr = scr_pool.tile([P, D], f16, tag="scr")
                    nc.vector.scalar_tensor_tensor(
                        out=scr,
                        in0=xn[:, i, :],
                        scalar=1.0,
                        in1=projbc[b],
                        op0=mybir.AluOpType.mult,
                        op1=mybir.AluOpType.mult,
                        accum_out=eij_n[:, b, i:i + 1],
                    )
                # paired DVE mul + ACT accum path: i = N_STT..NN-1
                i = N_STT
                while i < NN:
                    w = min(2, NN - i)
                    scr2 = scr_pool.tile([P, 2, D], f16, tag="scr2", bufs=4)
                    pbc2 = bass_mod.AP(
                        tensor=projbc[b].tensor,
                        offset=projbc[b].offset,
                        ap=[projbc[b].ap[0], [0, w]] + projbc[b].ap[1:])
                    nc.vector.tensor_mul(scr2[:, 0:w, :],
                                         xn[:, i:i + w, :], pbc2)
                    for k in range(w):
                        dump = scr_pool.tile([P, D], f16, tag="dump", bufs=4)
                        nc.scalar.activation(
                            dump, scr2[:, k, :],
                            mybir.ActivationFunctionType.Copy,
                            accum_out=eij_n[:, b, i + k:i + k + 1])
                    i += w
                # per-batch epilogue: tanh/exp/mask/reduce (pipelined)
                nc.scalar.activation(th[:, b, 0:NT], eij_ps[:, b, :],
                                     mybir.ActivationFunctionType.Tanh)
                nc.scalar.activation(th[:, b, NT:J], eij_n[:, b, :],
                                     mybir.ActivationFunctionType.Tanh)
                nc.scalar.activation(ex[:, b, :], th[:, b, :],
                                     mybir.ActivationFunctionType.Exp)
                nc.vector.tensor_mul(au[:, b, :], ex[:, b, :],
                                     mask_all[:, b, :])
                nc.vector.reduce_sum(cs[:, b:b + 1], au[:, b, :],
                                     axis=mybir.AxisListType.X)

            # ---- tail: normalize, transpose, store ----
            tot_ps = psum_small.tile([1, BL], f32, tag="small")
            nc.tensor.matmul(tot_ps, lhsT=ones_col, rhs=cs,
                             start=True, stop=True)
            tot_sb = consts.tile([1, BL], f32)
            nc.scalar.activation(tot_sb, tot_ps,
                                 mybir.ActivationFunctionType.Identity,
                                 bias=eps_t, scale=1.0)
            rec = consts.tile([1, BL], f32)
            nc.vector.reciprocal(rec, tot_sb)
            rbc_ps = psum_small.tile([P, BL], f32, tag="small")
            nc.tensor.matmul(rbc_ps, lhsT=ones_row, rhs=rec,
                             start=True, stop=True)
            rbc_sb = consts.tile([P, BL], f32)
            nc.scalar.copy(rbc_sb, rbc_ps)
            an = consts.tile([P, BL, J], f32)
            rbc_bc = bass_mod.AP(
                tensor=rbc_sb.tensor, offset=rbc_sb.offset,
                ap=[rbc_sb.ap[0], rbc_sb.ap[1], [0, J]])
            nc.vector.tensor_mul(an, au, rbc_bc)
            at_ps = psum_small.tile([BL * J, P], f32, tag="small")
            nc.tensor.transpose(at_ps, an.rearrange("p b j -> p (b j)"),
                                identity32)
            an_t = consts.tile([BL * J, P], f32)
            nc.scalar.copy(an_t, at_ps)
            nc.sync.dma_start(
                out=out_ext.rearrange("b (j p) -> (b j) p", p=P), in_=an_t)

    nc.compile()
    return nc


def _get_nc():
    if "nc" not in _CACHE:
        _CACHE["nc"] = _build()
    return _CACHE["nc"]


def _in_maps(x, y, mask, M):
    x16 = np.asarray(x, dtype=np.float32).astype(np.float16)
    y16 = np.asarray(y, dtype=np.float32).astype(np.float16)
    MT16 = np.ascontiguousarray(np.asarray(M, dtype=np.float32)
                                .astype(np.float16).T)
    mk = np.asarray(mask, dtype=np.int32).astype(np.float32)
    maps = []
    for i in range(NCORES):
        xs = x16[i * BL:(i + 1) * BL]
        xt = np.ascontiguousarray(xs[:, :ST, :].transpose(0, 2, 1))
        xn = np.ascontiguousarray(xs[:, ST:, :])
        mkc = mk[i * BL:(i + 1) * BL].reshape(BL, J, P).transpose(2, 0, 1)
        maps.append({
            "xt16": xt,
            "xn16": xn,
            "yT16": np.ascontiguousarray(y16[i * BL:(i + 1) * BL].T),
            "MT16": MT16,
            "maskT": np.ascontiguousarray(mkc),
        })
    return maps


def kernel(x, y, mask, M, **_ignored):
    from concourse.bass_utils import run_bass_kernel_spmd

    nc = _get_nc()
    res = run_bass_kernel_spmd(nc, _in_maps(x, y, mask, M),
                               core_ids=list(range(NCORES)))
    out = np.concatenate([res.results[i]["out"] for i in range(NCORES)],
                         axis=0)
    return out.astype(np.float32)


# revision 7
# speedup vs baseline: 1.1349x; 1.1349x over previous
"""Trainium2 Bass kernel for masked attention scoring (sparse_attention).

Computes, per batch b:
    proj = y @ M^T                      # [B, D]
    eij  = tanh(einsum('bsd,bd->bs', x, proj))
    a    = exp(eij) * mask
    a    = a / (sum_s a + EPS)

Sharding: data-parallel over batch B=32 across 8 NeuronCores (4 batches
per core). M is replicated; all reductions stay local per shard.

v3 design (memory-bound; measured x-stream rate ~390 GB/s/core):
  - x ships f16, split per batch into a host-TRANSPOSED part xt (j-chunks
    0..NT-1, laid out [e, s] so TensorE ingests it as 128x128 LDWEIGHTS
    tiles against a projT column; eij accumulates in PSUM on 128
    partitions) and a natural part xn (chunks NT..15) for the DVE
    STT / DVE-mul+ACT-accum path. Every engine stays under the DMA
    stream time so the queues run at their free rate.
  - DMA order puts xn (slow consumers) first and xt (PE consumes at
    stream rate) last, with the final piece small, so the post-stream
    tail is ~2us. The whole x shard is SBUF-resident; no pool recycling.
  - No cross-engine interlocks mid-stream: main-pass ops only, then
    per-batch tanh/exp, then one batched mask-mul/reduce/normalize tail.
  - mask ships host-pre-transposed [P, BL, J] f32; PE clock pre-warmed
    during the M DMA; proj GEMM + projT transposes + partition
    broadcasts all finish on PE before the x stream needs them.
"""

import os
import sys

import numpy as np

for _p in ("/opt/trn_rl_repo",):
    if os.path.isdir(_p) and _p not in sys.path:
        sys.path.insert(0, _p)

B, S, D = 32, 2048, 1024
NCORES = 8
BL = B // NCORES        # batches per core
P = 128                 # SBUF partitions
J = S // P              # 16 s-chunks of 128 per batch
DC = D // P             # 8 e-chunks of 128
NT = 9                  # PE-path s-chunks per batch (j = 0..NT-1)
NN = J - NT             # natural-path s-chunks (j = NT..J-1)
N_STT = 3               # of NN: chunks done as fused STT on DVE
ST = NT * P
SN = NN * P
EPS = 1e-7

_CACHE = {}


def _build():
    import concourse.bacc as bacc
    import concourse.bass as bass_mod
    import concourse.tile as tile
    from concourse import mybir
    from concourse.masks import make_identity

    f32 = mybir.dt.float32
    f16 = mybir.dt.float16

    nc = bacc.Bacc("TRN2", target_bir_lowering=False, debug=False,
                   num_devices=NCORES)

    xt_ext = nc.dram_tensor("xt16", [BL, D, ST], f16, kind="ExternalInput").ap()
    xn_ext = nc.dram_tensor("xn16", [BL, SN, D], f16, kind="ExternalInput").ap()
    y_ext = nc.dram_tensor("yT16", [D, BL], f16, kind="ExternalInput").ap()
    m_ext = nc.dram_tensor("MT16", [D, D], f16, kind="ExternalInput").ap()
    mk_ext = nc.dram_tensor("maskT", [P, BL, J], f32, kind="ExternalInput").ap()
    out_ext = nc.dram_tensor("out", [BL, S], f32, kind="ExternalOutput").ap()

    with tile.TileContext(nc) as tc:
        with (
            tc.tile_pool(name="consts", bufs=1) as consts,
            tc.tile_pool(name="psum_proj", bufs=1, space="PSUM") as psum_proj,
            tc.tile_pool(name="psum_eij", bufs=1, space="PSUM") as psum_eij,
            tc.tile_pool(name="psum_pb", bufs=2, space="PSUM") as psum_pb,
            tc.tile_pool(name="psum_small", bufs=1, space="PSUM") as psum_small,
            tc.tile_pool(name="scr", bufs=6) as scr_pool,
        ):
            identity16 = consts.tile([P, P], f16)
            make_identity(nc, identity16)
            identity32 = consts.tile([P, P], f32)
            make_identity(nc, identity32)
            ones_col = consts.tile([P, 1], f32)
            nc.vector.memset(ones_col, 1.0)
            ones_row = consts.tile([1, P], f32)
            nc.vector.memset(ones_row, 1.0)
            eps_t = consts.tile([1, 1], f32)
            nc.vector.memset(eps_t, EPS)

            # ---- M^T ships pre-transposed f16 (contiguous 2KB rows) ----
            mtsb = consts.tile([P, DC, D], f16)
            m_src = m_ext.rearrange("(dc p) e -> p dc e", p=P)
            nc.sync.dma_start(out=mtsb[:, 0:DC // 2, :],
                              in_=m_src[:, 0:DC // 2, :])
            nc.sync.dma_start(out=mtsb[:, DC // 2:, :],
                              in_=m_src[:, DC // 2:, :])

            # warm the PE clock (1.2 -> 2.4 GHz needs ~3us sustained)
            warm_ps = psum_small.tile([P, P], f16, tag="small")
            for _ in range(16):
                nc.tensor.transpose(warm_ps, identity16, identity16)

            # ---- small inputs ----
            yT = consts.tile([P, DC, BL], f16)
            nc.sync.dma_start(
                out=yT, in_=y_ext.rearrange("(dc p) b -> p dc b", p=P))
            mask_all = consts.tile([P, BL, J], f32)
            nc.sync.dma_start(out=mask_all, in_=mk_ext)

            # ---- x DMAs: xn (slow consumers) first, xt last ----
            xt_tiles = []
            xn_tiles = []
            for b in range(BL):
                xt_tiles.append(consts.tile([P, DC, ST], f16, name=f"xt{b}"))
                xn_tiles.append(consts.tile([P, NN, D], f16, name=f"xn{b}"))
            for b in range(BL):
                xn_src = xn_ext[b].rearrange("(i p) e -> p i e", p=P)
                nc.sync.dma_start(out=xn_tiles[b][:, 0:N_STT, :],
                                  in_=xn_src[:, 0:N_STT, :])
                nc.sync.dma_start(out=xn_tiles[b][:, N_STT:, :],
                                  in_=xn_src[:, N_STT:, :])
            # split xt by s-columns so PE can run whole (8-matmul
            # contiguous) accumulation groups per piece; last piece small
            JSPLIT = 5
            for b in range(BL):
                xt_src = xt_ext[b].rearrange("(ec p) s -> p ec s", p=P)
                nc.sync.dma_start(out=xt_tiles[b][:, :, 0:JSPLIT * P],
                                  in_=xt_src[:, :, 0:JSPLIT * P])
                nc.sync.dma_start(out=xt_tiles[b][:, :, JSPLIT * P:],
                                  in_=xt_src[:, :, JSPLIT * P:])

            # ---- proj[b, e] = sum_d y[b, d] * M[e, d]  (PSUM f32) ----
            proj_ps = psum_proj.tile([BL, D], f32)
            for dc in range(DC):
                for eh in range(2):
                    nc.tensor.matmul(
                        proj_ps[:, eh * 512:(eh + 1) * 512],
                        lhsT=yT[:, dc, :],
                        rhs=mtsb[:, dc, eh * 512:(eh + 1) * 512],
                        start=(dc == 0),
                        stop=(dc == DC - 1),
                    )
            proj_sb = consts.tile([BL, D], f16)
            nc.scalar.copy(proj_sb[:, 0:512], proj_ps[:, 0:512])
            nc.scalar.copy(proj_sb[:, 512:], proj_ps[:, 512:])

            # ---- projT[p, ec, b] = proj[b, ec*128+p] via PE transposes ----
            projT = consts.tile([P, DC, BL], f16)
            for ec in range(DC):
                tp_ps = psum_small.tile([P, BL], f16, tag="ptr", bufs=2)
                nc.tensor.transpose(
                    tp_ps, proj_sb[:, ec * P:(ec + 1) * P],
                    identity16[:BL, :BL])
                nc.scalar.copy(projT[:, ec, :], tp_ps)

            # ---- broadcast proj rows across partitions (DVE path) ----
            projbc = []
            for b in range(BL):
                sel = consts.tile([BL, P], f16, name=f"sel{b}")
                nc.gpsimd.memset(sel, 0.0)
                nc.gpsimd.affine_select(
                    out=sel, in_=sel,
                    compare_op=mybir.AluOpType.not_equal,
                    fill=1.0, base=-b,
                    pattern=[[0, P]], channel_multiplier=1)
                pb = consts.tile([P, D], f16, name=f"projbc{b}")
                for eh in range(2):
                    pb_ps = psum_pb.tile([P, 512], f32, tag="pbps")
                    nc.tensor.matmul(
                        pb_ps,
                        lhsT=sel,
                        rhs=proj_sb[:, eh * 512:(eh + 1) * 512],
                        start=True, stop=True)
                    if eh == 0:
                        nc.vector.tensor_copy(pb[:, eh * 512:(eh + 1) * 512],
                                              pb_ps)
                    else:
                        nc.scalar.copy(pb[:, eh * 512:(eh + 1) * 512], pb_ps)
                projbc.append(pb)

            # ---- main pass: DVE/ACT on xn chunks ----
            eij_ps = psum_eij.tile([P, BL, NT], f32)
            eij_n = consts.tile([P, BL, NN], f32)
            th = consts.tile([P, BL, J], f32)
            ex = consts.tile([P, BL, J], f32)

            for b in range(BL):
                xn = xn_tiles[b]
                for i in range(N_STT):
                    scr = scr_pool.tile([P, D], f16, tag="scr")
                    nc.vector.scalar_tensor_tensor(
                        out=scr,
                        in0=xn[:, i, :],
                        scalar=1.0,
                        in1=projbc[b],
                        op0=mybir.AluOpType.mult,
                        op1=mybir.AluOpType.mult,
                        accum_out=eij_n[:, b, i:i + 1],
                    )
                i = N_STT
                while i < NN:
                    w = min(2, NN - i)
                    scr2 = scr_pool.tile([P, 2, D], f16, tag="scr2", bufs=4)
                    pbc2 = bass_mod.AP(
                        tensor=projbc[b].tensor,
                        offset=projbc[b].offset,
                        ap=[projbc[b].ap[0], [0, w]] + projbc[b].ap[1:])
                    nc.vector.tensor_mul(scr2[:, 0:w, :],
                                         xn[:, i:i + w, :], pbc2)
                    for k in range(w):
                        dump = scr_pool.tile([P, D], f16, tag="dump", bufs=4)
                        nc.scalar.activation(
                            dump, scr2[:, k, :],
                            mybir.ActivationFunctionType.Copy,
                            accum_out=eij_n[:, b, i + k:i + k + 1])
                    i += w

            # ---- main pass: PE on xt chunks (contiguous 8-matmul groups) ----
            for b in range(BL):
                xt = xt_tiles[b]
                for j in range(NT):
                    for ec in range(DC):
                        nc.tensor.matmul(
                            eij_ps[:, b, j:j + 1],
                            lhsT=xt[:, ec, j * P:(j + 1) * P],
                            rhs=projT[:, ec, b:b + 1],
                            start=(ec == 0),
                            stop=(ec == DC - 1),
                        )

            # ---- tanh/exp per batch (ACT), in PE completion order ----
            for b in range(BL):
                nc.scalar.activation(th[:, b, NT:J], eij_n[:, b, :],
                                     mybir.ActivationFunctionType.Tanh)
                nc.scalar.activation(th[:, b, 0:NT], eij_ps[:, b, :],
                                     mybir.ActivationFunctionType.Tanh)
                nc.scalar.activation(ex[:, b, :], th[:, b, :],
                                     mybir.ActivationFunctionType.Exp)

            # ---- batched tail: mask, reduce, normalize, store ----
            au = consts.tile([P, BL, J], f32)
            nc.vector.tensor_mul(au, ex, mask_all)
            cs = consts.tile([P, BL], f32)
            nc.vector.reduce_sum(cs, au, axis=mybir.AxisListType.X)
            tot_ps = psum_small.tile([1, BL], f32, tag="small")
            nc.tensor.matmul(tot_ps, lhsT=ones_col, rhs=cs,
                             start=True, stop=True)
            tot_sb = consts.tile([1, BL], f32)
            nc.scalar.activation(tot_sb, tot_ps,
                                 mybir.ActivationFunctionType.Identity,
                                 bias=eps_t, scale=1.0)
            rec = consts.tile([1, BL], f32)
            nc.vector.reciprocal(rec, tot_sb)
            rbc_ps = psum_small.tile([P, BL], f32, tag="small")
            nc.tensor.matmul(rbc_ps, lhsT=ones_row, rhs=rec,
                             start=True, stop=True)
            rbc_sb = consts.tile([P, BL], f32)
            nc.scalar.copy(rbc_sb, rbc_ps)
            an = consts.tile([P, BL, J], f32)
            rbc_bc = bass_mod.AP(
                tensor=rbc_sb.tensor, offset=rbc_sb.offset,
                ap=[rbc_sb.ap[0], rbc_sb.ap[1], [0, J]])
            nc.vector.tensor_mul(an, au, rbc_bc)
            at_ps = psum_small.tile([BL * J, P], f32, tag="small")
            nc.tensor.transpose(at_ps, an.rearrange("p b j -> p (b j)"),
                                identity32)
            an_t = consts.tile([BL * J, P], f32)
            nc.scalar.copy(an_t, at_ps)
            nc.sync.dma_start(
                out=out_ext.rearrange("b (j p) -> (b j) p", p=P), in_=an_t)

    nc.compile()
    return nc


def _get_nc():
    if "nc" not in _CACHE:
        _CACHE["nc"] = _build()
    return _CACHE["nc"]


def _in_maps(x, y, mask, M):
    x16 = np.asarray(x, dtype=np.float32).astype(np.float16)
    y16 = np.asarray(y, dtype=np.float32).astype(np.float16)
    MT16 = np.ascontiguousarray(np.asarray(M, dtype=np.float32)
                                .astype(np.float16).T)
    mk = np.asarray(mask, dtype=np.int32).astype(np.float32)
    maps = []
    for i in range(NCORES):
        xs = x16[i * BL:(i + 1) * BL]
        xt = np.ascontiguousarray(xs[:, :ST, :].transpose(0, 2, 1))
        xn = np.ascontiguousarray(xs[:, ST:, :])
        mkc = mk[i * BL:(i + 1) * BL].reshape(BL, J, P).transpose(2, 0, 1)
        maps.append({
            "xt16": xt,
            "xn16": xn,
            "yT16": np.ascontiguousarray(y16[i * BL:(i + 1) * BL].T),
            "MT16": MT16,
            "maskT": np.ascontiguousarray(mkc),
        })
    return maps


def kernel(x, y, mask, M, **_ignored):
    from concourse.bass_utils import run_bass_kernel_spmd

    nc = _get_nc()
    res = run_bass_kernel_spmd(nc, _in_maps(x, y, mask, M),
                               core_ids=list(range(NCORES)))
    out = np.concatenate([res.results[i]["out"] for i in range(NCORES)],
                         axis=0)
    return out.astype(np.float32)


# revision 10
# speedup vs baseline: 1.1458x; 1.0096x over previous
"""Trainium2 Bass kernel for masked attention scoring (sparse_attention).

Computes, per batch b:
    proj = y @ M^T                      # [B, D]
    eij  = tanh(einsum('bsd,bd->bs', x, proj))
    a    = exp(eij) * mask
    a    = a / (sum_s a + EPS)

Sharding: data-parallel over batch B=32 across 8 NeuronCores (4 batches
per core). M is replicated; all reductions stay local per shard.

v3 design (memory-bound; measured x-stream rate ~390 GB/s/core):
  - x ships f16, split per batch into a host-TRANSPOSED part xt (j-chunks
    0..NT-1, laid out [e, s] so TensorE ingests it as 128x128 LDWEIGHTS
    tiles against a projT column; eij accumulates in PSUM on 128
    partitions) and a natural part xn (chunks NT..15) for the DVE
    STT / DVE-mul+ACT-accum path. Every engine stays under the DMA
    stream time so the queues run at their free rate.
  - DMA order puts xn (slow consumers) first and xt (PE consumes at
    stream rate) last, with the final piece small, so the post-stream
    tail is ~2us. The whole x shard is SBUF-resident; no pool recycling.
  - No cross-engine interlocks mid-stream: main-pass ops only, then
    per-batch tanh/exp, then one batched mask-mul/reduce/normalize tail.
  - mask ships host-pre-transposed [P, BL, J] f32; PE clock pre-warmed
    during the M DMA; proj GEMM + projT transposes + partition
    broadcasts all finish on PE before the x stream needs them.
"""

import os
import sys

import numpy as np

for _p in ("/opt/trn_rl_repo",):
    if os.path.isdir(_p) and _p not in sys.path:
        sys.path.insert(0, _p)

B, S, D = 32, 2048, 1024
NCORES = 8
BL = B // NCORES        # batches per core
P = 128                 # SBUF partitions
J = S // P              # 16 s-chunks of 128 per batch
DC = D // P             # 8 e-chunks of 128
NT = 9                  # PE-path s-chunks per batch (j = 0..NT-1)
NN = J - NT             # natural-path s-chunks (j = NT..J-1)
N_STT = 3               # of NN: chunks done as fused STT on DVE
ST = NT * P
SN = NN * P
EPS = 1e-7

_CACHE = {}


def _build():
    import concourse.bacc as bacc
    import concourse.bass as bass_mod
    import concourse.tile as tile
    from concourse import mybir
    from concourse.masks import make_identity

    f32 = mybir.dt.float32
    f16 = mybir.dt.float16

    nc = bacc.Bacc("TRN2", target_bir_lowering=False, debug=False,
                   num_devices=NCORES)

    xt_ext = nc.dram_tensor("xt16", [BL, D, ST], f16, kind="ExternalInput").ap()
    xn_ext = nc.dram_tensor("xn16", [BL, SN, D], f16, kind="ExternalInput").ap()
    y_ext = nc.dram_tensor("yT16", [D, BL], f16, kind="ExternalInput").ap()
    m_ext = nc.dram_tensor("MT16", [D, D], f16, kind="ExternalInput").ap()
    mk_ext = nc.dram_tensor("maskT", [P, BL, J], f32, kind="ExternalInput").ap()
    out_ext = nc.dram_tensor("out", [BL, S], f32, kind="ExternalOutput").ap()

    with tile.TileContext(nc) as tc:
        with (
            tc.tile_pool(name="consts", bufs=1) as consts,
            tc.tile_pool(name="psum_proj", bufs=1, space="PSUM") as psum_proj,
            tc.tile_pool(name="psum_eij", bufs=1, space="PSUM") as psum_eij,
            tc.tile_pool(name="psum_eij2", bufs=1, space="PSUM") as psum_eij2,
            tc.tile_pool(name="psum_pb", bufs=2, space="PSUM") as psum_pb,
            tc.tile_pool(name="psum_small", bufs=1, space="PSUM") as psum_small,
            tc.tile_pool(name="scr", bufs=6) as scr_pool,
        ):
            identity16 = consts.tile([P, P], f16)
            make_identity(nc, identity16)
            identity32 = consts.tile([P, P], f32)
            make_identity(nc, identity32)
            ones_col = consts.tile([P, 1], f32)
            nc.vector.memset(ones_col, 1.0)
            ones_row = consts.tile([1, P], f32)
            nc.vector.memset(ones_row, 1.0)
            eps_t = consts.tile([1, 1], f32)
            nc.vector.memset(eps_t, EPS)

            # ---- M^T ships pre-transposed f16 (contiguous 2KB rows) ----
            mtsb = consts.tile([P, DC, D], f16)
            m_src = m_ext.rearrange("(dc p) e -> p dc e", p=P)
            nc.sync.dma_start(out=mtsb[:, 0:DC // 2, :],
                              in_=m_src[:, 0:DC // 2, :])
            nc.sync.dma_start(out=mtsb[:, DC // 2:, :],
                              in_=m_src[:, DC // 2:, :])

            # warm the PE clock (1.2 -> 2.4 GHz needs ~3us sustained)
            warm_ps = psum_small.tile([P, P], f16, tag="small")
            for _ in range(16):
                nc.tensor.transpose(warm_ps, identity16, identity16)

            # ---- small inputs ----
            yT = consts.tile([P, DC, BL], f16)
            nc.sync.dma_start(
                out=yT, in_=y_ext.rearrange("(dc p) b -> p dc b", p=P))
            mask_all = consts.tile([P, BL, J], f32)
            nc.sync.dma_start(out=mask_all, in_=mk_ext)

            # ---- x DMAs: xn (slow consumers) first, xt last ----
            xt_tiles = []
            xn_tiles = []
            for b in range(BL):
                xt_tiles.append(consts.tile([P, DC, ST], f16, name=f"xt{b}"))
                xn_tiles.append(consts.tile([P, NN, D], f16, name=f"xn{b}"))
            for b in range(BL):
                xn_src = xn_ext[b].rearrange("(i p) e -> p i e", p=P)
                nc.sync.dma_start(out=xn_tiles[b][:, 0:N_STT, :],
                                  in_=xn_src[:, 0:N_STT, :])
                nc.sync.dma_start(out=xn_tiles[b][:, N_STT:, :],
                                  in_=xn_src[:, N_STT:, :])
            # split xt by e-chunk halves: keeps full 2.3KB descriptor rows
            for b in range(BL):
                xt_src = xt_ext[b].rearrange("(ec p) s -> p ec s", p=P)
                nc.sync.dma_start(out=xt_tiles[b][:, 0:4, :],
                                  in_=xt_src[:, 0:4, :])
                nc.sync.dma_start(out=xt_tiles[b][:, 4:, :],
                                  in_=xt_src[:, 4:, :])

            # ---- proj[b, e] = sum_d y[b, d] * M[e, d]  (PSUM f32) ----
            proj_ps = psum_proj.tile([BL, D], f32)
            for dc in range(DC):
                for eh in range(2):
                    nc.tensor.matmul(
                        proj_ps[:, eh * 512:(eh + 1) * 512],
                        lhsT=yT[:, dc, :],
                        rhs=mtsb[:, dc, eh * 512:(eh + 1) * 512],
                        start=(dc == 0),
                        stop=(dc == DC - 1),
                    )
            proj_sb = consts.tile([BL, D], f16)
            nc.scalar.copy(proj_sb[:, 0:512], proj_ps[:, 0:512])
            nc.scalar.copy(proj_sb[:, 512:], proj_ps[:, 512:])

            # ---- projT[p, ec, b] = proj[b, ec*128+p] via PE transposes ----
            projT = consts.tile([P, DC, BL], f16)
            for ec in range(DC):
                tp_ps = psum_small.tile([P, BL], f16, tag="ptr", bufs=1)
                nc.tensor.transpose(
                    tp_ps, proj_sb[:, ec * P:(ec + 1) * P],
                    identity16[:BL, :BL])
                nc.scalar.copy(projT[:, ec, :], tp_ps)

            # ---- broadcast proj rows across partitions (DVE path) ----
            projbc = []
            for b in range(BL):
                sel = consts.tile([BL, P], f16, name=f"sel{b}")
                nc.gpsimd.memset(sel, 0.0)
                nc.gpsimd.affine_select(
                    out=sel, in_=sel,
                    compare_op=mybir.AluOpType.not_equal,
                    fill=1.0, base=-b,
                    pattern=[[0, P]], channel_multiplier=1)
                pb = consts.tile([P, D], f16, name=f"projbc{b}")
                for eh in range(2):
                    pb_ps = psum_pb.tile([P, 512], f32, tag="pbps")
                    nc.tensor.matmul(
                        pb_ps,
                        lhsT=sel,
                        rhs=proj_sb[:, eh * 512:(eh + 1) * 512],
                        start=True, stop=True)
                    if eh == 0:
                        nc.vector.tensor_copy(pb[:, eh * 512:(eh + 1) * 512],
                                              pb_ps)
                    else:
                        nc.scalar.copy(pb[:, eh * 512:(eh + 1) * 512], pb_ps)
                projbc.append(pb)

            # ---- main pass: DVE/ACT on xn chunks ----
            eij_ps = psum_eij.tile([P, BL, NT], f32)
            eij_ps2 = psum_eij2.tile([P, BL, NT], f32)
            eij_a = consts.tile([P, BL, NT], f32)
            eij_all = consts.tile([P, BL, J], f32)
            th = consts.tile([P, BL, J], f32)
            ex = consts.tile([P, BL, J], f32)

            for b in range(BL):
                xn = xn_tiles[b]
                for i in range(N_STT):
                    scr = scr_pool.tile([P, D], f16, tag="scr")
                    nc.vector.scalar_tensor_tensor(
                        out=scr,
                        in0=xn[:, i, :],
                        scalar=1.0,
                        in1=projbc[b],
                        op0=mybir.AluOpType.mult,
                        op1=mybir.AluOpType.mult,
                        accum_out=eij_all[:, b, NT + i:NT + i + 1],
                    )
                i = N_STT
                while i < NN:
                    w = min(2, NN - i)
                    scr2 = scr_pool.tile([P, 2, D], f16, tag="scr2", bufs=4)
                    pbc2 = bass_mod.AP(
                        tensor=projbc[b].tensor,
                        offset=projbc[b].offset,
                        ap=[projbc[b].ap[0], [0, w]] + projbc[b].ap[1:])
                    nc.vector.tensor_mul(scr2[:, 0:w, :],
                                         xn[:, i:i + w, :], pbc2)
                    for k in range(w):
                        dump = scr_pool.tile([P, D], f16, tag="dump", bufs=4)
                        nc.scalar.activation(
                            dump, scr2[:, k, :],
                            mybir.ActivationFunctionType.Copy,
                            accum_out=eij_all[:, b, NT + i + k:NT + i + k + 1])
                    i += w

            # ---- main pass: PE on xt chunks. Two phases per batch (one
            # per e-chunk half / DMA piece); each column's 4-matmul
            # accumulation group is contiguous within its bank.
            for b in range(BL):
                xt = xt_tiles[b]
                for (e0, e1, ps) in ((0, 4, eij_ps), (4, 8, eij_ps2)):
                    for j in range(NT):
                        for ec in range(e0, e1):
                            nc.tensor.matmul(
                                ps[:, b, j:j + 1],
                                lhsT=xt[:, ec, j * P:(j + 1) * P],
                                rhs=projT[:, ec, b:b + 1],
                                start=(ec == e0),
                                stop=(ec == e1 - 1),
                            )
                # phase-A bank to SBUF (hidden under the stream); the
                # phase-B bank merges in with a single-PSUM-operand add
                nc.scalar.copy(eij_a[:, b, :], eij_ps[:, b, :])

            # ---- merge PE phase banks, then tanh/exp per batch ----
            for b in range(BL):
                nc.vector.tensor_add(eij_all[:, b, 0:NT],
                                     eij_a[:, b, :], eij_ps2[:, b, :])
            for b in range(BL):
                nc.scalar.activation(th[:, b, :], eij_all[:, b, :],
                                     mybir.ActivationFunctionType.Tanh)
                nc.scalar.activation(ex[:, b, :], th[:, b, :],
                                     mybir.ActivationFunctionType.Exp)

            # ---- batched tail: mask, reduce, normalize, store ----
            au = consts.tile([P, BL, J], f32)
            nc.vector.tensor_mul(au, ex, mask_all)
            cs = consts.tile([P, BL], f32)
            nc.vector.reduce_sum(cs, au, axis=mybir.AxisListType.X)
            tot_ps = psum_small.tile([1, BL], f32, tag="small")
            nc.tensor.matmul(tot_ps, lhsT=ones_col, rhs=cs,
                             start=True, stop=True)
            tot_sb = consts.tile([1, BL], f32)
            nc.scalar.activation(tot_sb, tot_ps,
                                 mybir.ActivationFunctionType.Identity,
                                 bias=eps_t, scale=1.0)
            rec = consts.tile([1, BL], f32)
            nc.vector.reciprocal(rec, tot_sb)
            rbc_ps = psum_small.tile([P, BL], f32, tag="small")
            nc.tensor.matmul(rbc_ps, lhsT=ones_row, rhs=rec,
                             start=True, stop=True)
            rbc_sb = consts.tile([P, BL], f32)
            nc.scalar.copy(rbc_sb, rbc_ps)
            an = consts.tile([P, BL, J], f32)
            rbc_bc = bass_mod.AP(
                tensor=rbc_sb.tensor, offset=rbc_sb.offset,
                ap=[rbc_sb.ap[0], rbc_sb.ap[1], [0, J]])
            nc.vector.tensor_mul(an, au, rbc_bc)
            at_ps = psum_small.tile([BL * J, P], f32, tag="small")
            nc.tensor.transpose(at_ps, an.rearrange("p b j -> p (b j)"),
                                identity32)
            an_t = consts.tile([BL * J, P], f32)
            nc.scalar.copy(an_t, at_ps)
            nc.sync.dma_start(
                out=out_ext.rearrange("b (j p) -> (b j) p", p=P), in_=an_t)

    nc.compile()
    return nc


def _get_nc():
    if "nc" not in _CACHE:
        _CACHE["nc"] = _build()
    return _CACHE["nc"]


def _in_maps(x, y, mask, M):
    x16 = np.asarray(x, dtype=np.float32).astype(np.float16)
    y16 = np.asarray(y, dtype=np.float32).astype(np.float16)
    MT16 = np.ascontiguousarray(np.asarray(M, dtype=np.float32)
                                .astype(np.float16).T)
    mk = np.asarray(mask, dtype=np.int32).astype(np.float32)
    maps = []
    for i in range(NCORES):
        xs = x16[i * BL:(i + 1) * BL]
        xt = np.ascontiguousarray(xs[:, :ST, :].transpose(0, 2, 1))
        xn = np.ascontiguousarray(xs[:, ST:, :])
        mkc = mk[i * BL:(i + 1) * BL].reshape(BL, J, P).transpose(2, 0, 1)
        maps.append({
            "xt16": xt,
            "xn16": xn,
            "yT16": np.ascontiguousarray(y16[i * BL:(i + 1) * BL].T),
            "MT16": MT16,
            "maskT": np.ascontiguousarray(mkc),
        })
    return maps


def kernel(x, y, mask, M, **_ignored):
    from concourse.bass_utils import run_bass_kernel_spmd

    nc = _get_nc()
    res = run_bass_kernel_spmd(nc, _in_maps(x, y, mask, M),
                               core_ids=list(range(NCORES)))
    out = np.concatenate([res.results[i]["out"] for i in range(NCORES)],
                         axis=0)
    return out.astype(np.float32)
